# revision 22
# baseline (speedup 1.0000x reference)
"""Soft decision-tree layer (depth 4, 16 leaves) on 8 trn2 NeuronCores.

Sharding: 2-way data parallel (token halves) x 4-way expert parallel
(4 leaves per core).  Each core computes, for its 2048 tokens t and its
4 leaves l:  partial[t,:] = sum_l path_l(t) * (x[t] @ Wl[l] + bl[l]).
Host sums the 4 expert partials per token half.

Matmuls run in float32r (full PE rate at N=512, ~1e-4 input rounding);
accumulation is fp32 in PSUM/SBUF.

Per-core decision data is pre-sliced on the host so the SPMD program is
core-independent: a [1024, 16] matrix whose sigmoid columns are
  0..5   : nodes 0,1,2 (both choices)          -> level 0/1 path products
  6..9   : nodes 3+l, choice e0 (l = 0..3)     -> level-2 factor per leaf
  10..13 : nodes 7+4*e0+l, choice e1           -> level-3 factor per leaf
  14..15 : zero padding (unused)
path_l = P4_l * dec[6+l] * dec[10+l], where P4 comes from cols 0..5.
"""

import numpy as np

GEMM_DT = "float16"     # "float32r" | "float16" | "bfloat16"
B, S, H = 2, 2048, 1024
DP, EP = 2, 4            # data-parallel x expert-parallel = 8 cores
T = (B * S) // DP        # 2048 tokens per core
LPC = 16 // EP           # 4 leaves per core
NT = T // 128            # 16 token tiles per core
TG = 2                   # token groups (acc working set = 8 tiles)
TPG = NT // TG           # 8 token tiles per group
KC = H // 128            # 8 contraction chunks
ND = 16                  # decision columns (14 used + 2 pad)

_prog_cache = {}


def _enable_ldw_opt():
    """Let walrus dedup consecutive LDWEIGHTS with identical source APs.

    concourse hardcodes --enable-ldw-opt=false; our matmul stream issues
    each stationary twice (two 512-wide n-halves), so deduping the
    reload takes the PE from weight-load-bound (242ns/mm) to
    stream-bound (213ns/mm).
    """
    from concourse import bass_utils as bu

    if getattr(bu, "_ldw_opt_patched", False):
        return
    orig = bu.run_command

    def run_command(cmd, *a, **kw):
        if GEMM_DT == "float32r":
            cmd = [c.replace("--enable-ldw-opt=false", "--enable-ldw-opt=true")
                   if isinstance(c, str) else c for c in cmd]
        return orig(cmd, *a, **kw)

    bu.run_command = run_command
    bu._ldw_opt_patched = True


def _build_program():
    if "nc" in _prog_cache:
        return _prog_cache["nc"]

    from contextlib import ExitStack
    import concourse.bacc as bacc
    import concourse.tile as tile
    import concourse.mybir as mybir

    _enable_ldw_opt()

    f32 = mybir.dt.float32
    f32r = getattr(mybir.dt, GEMM_DT)
    MULT = mybir.AluOpType.mult
    ADD = mybir.AluOpType.add
    SIG = mybir.ActivationFunctionType.Sigmoid

    nc = bacc.Bacc("TRN2", target_bir_lowering=False, debug=False, num_devices=8)

    xt_d = nc.dram_tensor("xt", [H, T], f32r, kind="ExternalInput").ap()
    wl_d = nc.dram_tensor("wl", [LPC, H, H], f32r, kind="ExternalInput").ap()
    wd_d = nc.dram_tensor("wd", [H, ND], f32r, kind="ExternalInput").ap()
    bd_d = nc.dram_tensor("bd", [1, ND], f32r, kind="ExternalInput").ap()
    bl_d = nc.dram_tensor("bl", [1, LPC * H], f32r, kind="ExternalInput").ap()
    ones_d = nc.dram_tensor("ones", [1, 128], f32r, kind="ExternalInput").ap()
    out_d = nc.dram_tensor("out", [T, H], f32, kind="ExternalOutput").ap()

    with tile.TileContext(nc) as tc, ExitStack() as ctx:
        consts = ctx.enter_context(tc.tile_pool(name="consts", bufs=1))
        xt_pool = ctx.enter_context(tc.tile_pool(name="xt", bufs=1))
        wl_pool = ctx.enter_context(tc.tile_pool(name="wl", bufs=1))
        acc_pool = ctx.enter_context(tc.tile_pool(name="acc", bufs=1))
        dec_pool = ctx.enter_context(tc.tile_pool(name="dec", bufs=2))
        ps_pool = ctx.enter_context(tc.tile_pool(name="ps", bufs=8, space="PSUM"))

        # --- PE warmup: ~4us of matmuls on a memset tile, no DMA deps,
        #     so the HAM clock gate reaches 2.4GHz before real work ---
        if GEMM_DT != "float32r":   # DVE memset can't produce float32r
            warm = consts.tile([128, 512], f32r, tag="warm")
            nc.vector.memset(warm[:], 0.0)
            wps = ps_pool.tile([128, 512], f32, tag="ps", name="warmps")
            for i in range(20):
                nc.tensor.matmul(wps[:], warm[:, 0:128], warm[:],
                                 start=True, stop=True)

        # --- constants (tiny DMAs first so early PE work isn't queued
        #     behind the bulk transfers) ---
        ones = consts.tile([1, 128], f32r, tag="ones")
        nc.sync.dma_start(ones[:], ones_d[:, :])
        wd_sb = consts.tile([128, KC * ND], f32r, tag="wd")
        nc.sync.dma_start(
            wd_sb[:].rearrange("p (k n) -> p k n", k=KC),
            wd_d.rearrange("(k p) n -> p k n", p=128),
        )
        bd_sb = consts.tile([1, ND], f32r, tag="bd")
        nc.sync.dma_start(bd_sb[:], bd_d[:, :])
        bl_sb = consts.tile([1, LPC * H], f32r, tag="bl")
        nc.sync.dma_start(bl_sb[:], bl_d[:, :])

        # bd / bl broadcast to all 128 partitions via ones-vector matmul
        bdb = consts.tile([128, ND], f32, tag="bdb")
        bp = ps_pool.tile([128, 512], f32, tag="ps")
        nc.tensor.matmul(bp[:, 0:ND], ones[:], bd_sb[:], start=True, stop=True)
        nc.vector.tensor_copy(bdb[:], bp[:, 0:ND])
        blb = consts.tile([128, LPC * H], f32, tag="blb")
        for j in range(LPC * H // 512):
            bp = ps_pool.tile([128, 512], f32, tag="ps")
            nc.tensor.matmul(bp[:], ones[:], bl_sb[:, j * 512:(j + 1) * 512],
                             start=True, stop=True)
            nc.vector.tensor_copy(blb[:, j * 512:(j + 1) * 512], bp[:])

        # --- resident transposed activations, per (k-chunk, token group);
        #     group 1 chunks are queued later so they don't delay wl l0 ---
        xt = {}

        def load_xt(g):
            for k in range(KC):
                t_ = xt_pool.tile([128, T // TG], f32r, tag=f"xt{k}_{g}",
                                  name=f"xt{k}_{g}")
                nc.sync.dma_start(
                    t_[:], xt_d[k * 128:(k + 1) * 128,
                                g * (T // TG):(g + 1) * (T // TG)])
                xt[k, g] = t_

        load_xt(0)
        wl_res = {}
        for g in range(TG):
            dec_sb = dec_pool.tile([128, TPG * ND], f32, tag="dec")
            path = dec_pool.tile([128, TPG * LPC], f32, tag="path")
            accs = [acc_pool.tile([128, H], f32, tag=f"acc{t}",
                                  name=f"acc{t}_{g}")
                    for t in range(TPG)]

            def sig_path_init(t, dps):
                # sigmoid(dec + bd), this tile's 4 path columns, acc init
                tadd = dec_pool.tile([128, ND], f32, tag="tadd",
                                     name=f"tadd{t}_{g}")
                nc.vector.tensor_tensor(tadd[:], dps, bdb[:], op=ADD)
                dsl = dec_sb[:, t * ND:(t + 1) * ND]
                nc.scalar.activation(dsl, tadd[:], SIG)
                d3 = dsl.rearrange("p (n c) -> p n c", c=2)
                pt = path[:, t * LPC:(t + 1) * LPC]
                # P4[m] = P2[m%2] * dec[node 1+m%2, choice m//2]
                p4 = dec_pool.tile([128, 4], f32, tag="p4",
                                   name=f"p4_{t}_{g}")
                nc.vector.tensor_tensor(
                    p4[:, 0:2], dsl[:, 0:2], d3[:, 1:3, 0], op=MULT)
                nc.vector.tensor_tensor(
                    p4[:, 2:4], dsl[:, 0:2], d3[:, 1:3, 1], op=MULT)
                p4b = dec_pool.tile([128, 4], f32, tag="p4b",
                                    name=f"p4b_{t}_{g}")
                nc.vector.tensor_tensor(p4b[:], p4[:], dsl[:, 6:10], op=MULT)
                nc.vector.tensor_tensor(pt, p4b[:], dsl[:, 10:14], op=MULT)
                # init acc with leaf 0's path-weighted bias
                # (leaves 1-3 add theirs during their own pass)
                nc.vector.tensor_scalar(
                    accs[t][:], blb[:, 0:H], pt[:, 0:1], None, op0=MULT)

            def evict(t, l, ps_t, half):
                pcol = path[:, t * LPC + l:t * LPC + l + 1]
                o = half * 512
                nc.vector.scalar_tensor_tensor(
                    accs[t][:, o:o + 512], ps_t[:], pcol,
                    accs[t][:, o:o + 512], op0=MULT, op1=ADD)

            if g == 0:
                # Cold start: nothing is resident yet, so pace the PE by
                # the DMA stream.  Decisions consume only xt chunks
                # (k-outer), then leaf 0's n=0 half consumes wl chunks as
                # they land; the n=1 half then runs on resident data.
                # All leaves' weights fit in SBUF at 2 bytes/elem: load
                # every (l, k) chunk once, on the gpsimd queue, leaf 0
                # first.
                for ll in range(LPC):
                    for k in range(KC):
                        w = wl_pool.tile([128, H], f32r, tag=f"wl{ll}_{k}",
                                         name=f"wl{ll}_{k}")
                        nc.sync.dma_start(
                            w[:], wl_d[ll, k * 128:(k + 1) * 128, :])
                        wl_res[ll, k] = w
                wls = [wl_res[0, k] for k in range(KC)]
                dpss = [ps_pool.tile([128, 512], f32, tag="ps",
                                     name=f"dp{t}_0")
                        for t in range(TPG)]
                for k in range(KC):
                    for t in range(TPG):
                        nc.tensor.matmul(
                            dpss[t][:, 0:ND],
                            xt[k, 0][:, t * 128:(t + 1) * 128],
                            wd_sb[:, k * ND:(k + 1) * ND],
                            start=(k == 0), stop=(k == KC - 1))
                for t in range(TPG):
                    sig_path_init(t, dpss[t][:, 0:ND])
                # leaf 0, n=0: k-outer, 8 concurrent chains
                pss = [ps_pool.tile([128, 512], f32, tag="ps",
                                    name=f"pa{t}_0")
                       for t in range(TPG)]
                for k in range(KC):
                    for t in range(TPG):
                        nc.tensor.matmul(
                            pss[t][:], xt[k, 0][:, t * 128:(t + 1) * 128],
                            wls[k][:, 0:512],
                            start=(k == 0), stop=(k == KC - 1))
                for t in range(TPG):
                    evict(t, 0, pss[t], 0)
                # leaf 0, n=1: t-major on resident data
                for t in range(TPG):
                    psr = ps_pool.tile([128, 512], f32, tag="ps",
                                       name=f"pb{t}_0")
                    for k in range(KC):
                        nc.tensor.matmul(
                            psr[:], xt[k, 0][:, t * 128:(t + 1) * 128],
                            wls[k][:, 512:1024],
                            start=(k == 0), stop=(k == KC - 1))
                    evict(t, 0, psr, 1)
                l_range = range(1, LPC)
            else:
                l_range = range(LPC)

            for l in l_range:
                wls = [wl_res[l, k] for k in range(KC)]
                if g == 0 and l == 1:
                    load_xt(1)
                for t in range(TPG):
                    psl = ps_pool.tile([128, 512], f32, tag="ps",
                                       name=f"pl{l}_{t}_{g}")
                    psr = ps_pool.tile([128, 512], f32, tag="ps",
                                       name=f"pr{l}_{t}_{g}")
                    dps = None
                    if g > 0 and l == 0:
                        dps = ps_pool.tile([128, 512], f32, tag="ps",
                                           name=f"dp{t}_{g}")
                    for k in range(KC):
                        lhsT = xt[k, g][:, t * 128:(t + 1) * 128]
                        nc.tensor.matmul(psl[:], lhsT, wls[k][:, 0:512],
                                         start=(k == 0), stop=(k == KC - 1))
                        nc.tensor.matmul(psr[:], lhsT, wls[k][:, 512:1024],
                                         start=(k == 0), stop=(k == KC - 1))
                        if dps is not None:
                            # decision logits ride along on the same
                            # stationary (LDW deduped by walrus)
                            nc.tensor.matmul(
                                dps[:, 0:ND], lhsT,
                                wd_sb[:, k * ND:(k + 1) * ND],
                                start=(k == 0), stop=(k == KC - 1))
                    if dps is not None:
                        sig_path_init(t, dps[:, 0:ND])
                    if l == 1:
                        nc.vector.scalar_tensor_tensor(
                            accs[t][:], blb[:, H:2 * H],
                            path[:, t * LPC + 1:t * LPC + 2],
                            accs[t][:], op0=MULT, op1=ADD)
                    elif l == 2:
                        # leaf 3's bias rides here too, keeping the final
                        # leaf pass (the pipeline tail) DVE-light
                        for j in (2, 3):
                            nc.vector.scalar_tensor_tensor(
                                accs[t][:], blb[:, j * H:(j + 1) * H],
                                path[:, t * LPC + j:t * LPC + j + 1],
                                accs[t][:], op0=MULT, op1=ADD)
                    evict(t, l, psl, 0)
                    evict(t, l, psr, 1)
                    if l == LPC - 1:
                        r0 = (g * TPG + t) * 128
                        nc.sync.dma_start(out_d[r0:r0 + 128, :], accs[t][:])


    nc.compile()
    _prog_cache["nc"] = nc
    return nc


def _core_inputs(x, Wd, bd, Wl, bl):
    """Build the 8 per-core input dicts (host-side sharding)."""
    if GEMM_DT == "float16":
        cvt = np.float16
    elif GEMM_DT == "bfloat16":
        import ml_dtypes
        cvt = ml_dtypes.bfloat16
    else:
        cvt = np.float32
    x2 = np.ascontiguousarray(x, dtype=np.float32).reshape(B * S, H)
    Wd = np.asarray(Wd, dtype=np.float32)
    bd = np.asarray(bd, dtype=np.float32)
    Wl = np.ascontiguousarray(Wl, dtype=np.float32)
    bl = np.asarray(bl, dtype=np.float32)

    xts = [np.ascontiguousarray(x2[d * T:(d + 1) * T].T) for d in range(DP)]

    in_maps = []
    for c in range(8):
        d, e = c // EP, c % EP
        e1, e0 = e // 2, e % 2
        wd_c = np.zeros((H, ND), dtype=np.float32)
        bd_c = np.zeros((1, ND), dtype=np.float32)
        for n in range(3):                      # nodes 0,1,2 both choices
            wd_c[:, 2 * n:2 * n + 2] = Wd[n]
            bd_c[0, 2 * n:2 * n + 2] = bd[n]
        for l in range(4):
            wd_c[:, 6 + l] = Wd[3 + l, :, e0]   # level-2 factor
            bd_c[0, 6 + l] = bd[3 + l, e0]
            n3 = 7 + 4 * e0 + l                 # level-3 factor
            wd_c[:, 10 + l] = Wd[n3, :, e1]
            bd_c[0, 10 + l] = bd[n3, e1]
        in_maps.append({
            "xt": xts[d].astype(cvt),
            "wl": np.ascontiguousarray(Wl[LPC * e:LPC * (e + 1)]).astype(cvt),
            "wd": wd_c.astype(cvt),
            "bd": bd_c.astype(cvt),
            "bl": np.ascontiguousarray(
                bl[LPC * e:LPC * (e + 1)].reshape(1, LPC * H)).astype(cvt),
            "ones": np.ones((1, 128), dtype=cvt),
        })
    return in_maps


def kernel(x, Wd, bd, Wl, bl, _want_results=False):
    from concourse import bass_utils

    nc = _build_program()
    in_maps = _core_inputs(x, Wd, bd, Wl, bl)
    res = bass_utils.run_bass_kernel_spmd(nc, in_maps, list(range(8)))

    out = np.empty((DP, T, H), dtype=np.float32)
    for d in range(DP):
        s = np.zeros((T, H), dtype=np.float64)
        for e in range(EP):
            s += res.results[d * EP + e]["out"]
        out[d] = s.astype(np.float32)
    out = out.reshape(B, S, H)
    if _want_results:
        return out, res
    return out
